# revision 24
# baseline (speedup 1.0000x reference)
"""Trainium2 Bass kernel for nn_MinimalAdderNN.

Computation (see reference): a 10-digit ripple-carry adder over base-10 digit
tensors a, b of shape [1048576, 10] (int32, digits 0..9), using two lookup
tables built deterministically by setup_inputs(). For those structured tables
the output rows are exact one-hots:
    out[n, 1+p, k] = (k == (a[n,p] + b[n,p] + carry_in) % 10)
    out[n, 0,   k] = (k == final_carry)
so the kernel computes digits/carries arithmetically on the device:
    s = a + b  ->  carry chain via one tensor_tensor_scan (pad slots reset
    state between batch elements)  ->  t = s + carry_in  ->  d = t mod 10
    ->  one-hot = is_equal(value, k) materialized as int8 {0,1} split across
    the Vector, Scalar(Act) and GpSimd engines.

The full one-hot tensor (every output element's value) is computed on
device and DMA'd out as int8; the host-side gather step only converts the
storage format (int8 0/1 -> float32 0.0/1.0, an exact cast) while
concatenating the 8 per-core shards. Writing 1 byte instead of 4 per
element cuts the dominant HBM write traffic 4x.

Sharding: pure data-parallel over batch across 8 NeuronCores (131072 rows
per core); the tables are consumed host-side only (validated against the
expected structured tables).
"""
import sys

sys.path.insert(0, "/opt/trn_rl_repo")

import numpy as np

import concourse.bacc as bacc
import concourse.mybir as mybir
import concourse.tile as tile
from concourse.bass_utils import run_bass_kernel_spmd

BATCH = 1048576
D = 10
NCORES = 8
NPC = BATCH // NCORES  # 131072 rows per core
P = 128
NQ = NPC // P          # 1024 batch elems per partition
# Variable tile sizes (batch elems per partition per tile): small head/tail
# tiles shorten pipeline fill/drain; must sum to NQ.
QS = [64, 192, 192, 192, 192, 96, 64, 32]
PW = 11                # padded slots per batch elem (10 digits + 1 pad)
W = D + 1              # output positions per batch elem (leading + 10)
OW = W * D             # 110 output bytes per batch elem

# Engine split of the one-hot work over the 11 output positions w
# (w=0 leading carry, w=1..10 digits MSD..LSD). Each engine materializes
# o[:, :, w_range, k_range] = is_equal(val[w], k).
# One-hot lanes k are computed as whole planes: plane k holds
# [val[s,w] == k] for all (s,w). Output HBM layout is k-major
# [10, NPC, 11] fp16; the host merges planes into [n, w, k] f32
# (exact cast). Plane list split between DVE (4x tensor_scalar
# is_equal) and ScalarE (2-pass square/relu, fully contiguous).
ACT_PLANES = (7, 8, 9)     # k-planes computed by ScalarE
ALT_PLANE = 6              # ... plus this one on odd tiles

f32 = mybir.dt.float32
f16 = mybir.dt.float16
i8 = mybir.dt.int8

_CACHE = {}


def _expected_tables():
    next_carry = np.zeros((200, 2), dtype=np.float32)
    digit = np.zeros((200, 10), dtype=np.float32)
    for carry in (0, 1):
        for a_ in range(10):
            for b_ in range(10):
                idx = carry * 100 + a_ * 10 + b_
                total = a_ + b_ + carry
                next_carry[idx, total // 10] = 1.0
                digit[idx, total % 10] = 1.0
    return digit, next_carry


def _tables_are_structured(digit_table, carry_table):
    digit_exp, carry_exp = _expected_tables()
    if digit_table.shape != (200, 10) or carry_table.shape != (200, 2):
        return False
    if not np.array_equal(digit_table, digit_exp):
        return False
    # The reference only consumes argmax(carry_table[idx]); the fast path is
    # valid iff that argmax equals the arithmetic carry bit for every index.
    bits = np.argmax(carry_table, axis=1)
    return np.array_equal(bits, np.argmax(carry_exp, axis=1))


def _build_fast_nc():
    assert sum(QS) == NQ
    qmax = max(QS)
    nc = bacc.Bacc()
    ab_d = nc.dram_tensor("ab", [2, NPC, D], i8, kind="ExternalInput").ap()
    o_d = nc.dram_tensor("out", [D, NPC, W], f16, kind="ExternalOutput").ap()

    with tile.TileContext(nc) as tc:
        with tc.tile_pool(name="const", bufs=1) as cp, \
             tc.tile_pool(name="io", bufs=2) as iop, \
             tc.tile_pool(name="wk", bufs=2) as wp, \
             tc.tile_pool(name="ot", bufs=3) as op_:
            tens = cp.tile([P, 2], f16, tag="tens")
            nc.vector.memset(tens[:], 10.0)
            bias_t = cp.tile([P, D], f32, tag="bias")
            for k in range(D):
                nc.vector.memset(bias_t[:, k:k + 1], -float(k))

            r0 = 0
            for t_i, Q in enumerate(QS):
                ab_src = ab_d[:, r0:r0 + P * Q, :] \
                    .rearrange("z (p q) d -> p z (q d)", q=Q)
                o_dst = o_d[:, r0:r0 + P * Q, :] \
                    .rearrange("k (p q) w -> p k (q w)", q=Q)
                r0 += P * Q

                # ---- load (SWDGE cast int8 -> fp16 on the gpsimd ring)
                abt = iop.tile([P, 2 * qmax * D], f16, tag="ab")
                nc.gpsimd.dma_start(
                    abt[:, :2 * Q * D].rearrange("p (z f) -> p z f", z=2),
                    ab_src)
                ab4 = abt[:, :2 * Q * D].rearrange("p (z q d) -> p z q d",
                                                   z=2, d=D)
                a3 = ab4[:, 0]
                b3 = ab4[:, 1]

                # ---- s[q, j] = a[q, 9-j] + b[q, 9-j]  (LSD first), pads 0
                s_pad = wp.tile([P, qmax * PW], f16, tag="s")
                sp3 = s_pad[:, :Q * PW].rearrange("p (q e) -> p q e", e=PW)
                nc.gpsimd.memset(sp3[:, :, D:PW], 0.0)
                nc.gpsimd.tensor_tensor(sp3[:, :, 0:D], a3[:, :, ::-1],
                                        b3[:, :, ::-1], op=mybir.AluOpType.add)

                # ---- carry chain: scan written shifted by +1 so that
                # c3[q, j] = carry INTO digit j; c3[q, 10] = final carry.
                c = wp.tile([P, qmax * PW + 2], f16, tag="c")
                nc.gpsimd.memset(c[:, 0:1], 0.0)
                nc.vector.tensor_tensor_scan(
                    c[:, 1:1 + Q * PW], s_pad[:, :Q * PW],
                    tens[:, 0:1].broadcast_to([P, Q * PW]), 0.0,
                    op0=mybir.AluOpType.add, op1=mybir.AluOpType.is_ge)
                c3 = c[:, 0:Q * PW].rearrange("p (q e) -> p q e", e=PW)

                # ---- carry correction, flat+forward on DVE:
                # w[q,j] = carry_in(j) - 10*carry_out(j)  (j=0..9)
                # w[q,10] = final carry (c3[q,10] - 10*next-sample pad 0)
                cw = wp.tile([P, qmax * PW + 2], f16, tag="w")
                nc.vector.scalar_tensor_tensor(
                    cw[:, :Q * PW], c[:, 1:1 + Q * PW], -10.0,
                    c[:, 0:Q * PW],
                    op0=mybir.AluOpType.mult, op1=mybir.AluOpType.add)
                cw3 = cw[:, :Q * PW].rearrange("p (q e) -> p q e", e=PW)

                # ---- v[q, w] = value at output position w, CONTIGUOUS:
                #      w=0 -> final carry; w>=1 -> digit j=10-w:
                #      s[q,j] + carry_in(j) - 10*carry_out(j)
                tt = wp.tile([P, qmax * W], f16, tag="t")
                t3 = tt[:, :Q * W].rearrange("p (q e) -> p q e", e=W)
                nc.gpsimd.tensor_tensor(t3, sp3[:, :, 0:W][:, :, ::-1],
                                        cw3[:, :, 0:W][:, :, ::-1],
                                        op=mybir.AluOpType.add)

                v_flat = tt[:, :Q * W]

                # ---- one-hot planes
                ot = op_.tile([P, qmax * W * D], f16, tag="o")
                o5 = ot[:, :Q * W * D].rearrange("p (k f) -> p k f", k=D)

                act_planes = list(ACT_PLANES)
                if 1 <= t_i <= 4:
                    act_planes.append(ALT_PLANE)
                nap_t = len(act_planes)
                u = wp.tile([P, qmax * W * (len(ACT_PLANES) + 1)], f16,
                            tag="u")
                u3 = u[:, :Q * W * nap_t].rearrange("p (k f) -> p k f",
                                                    k=nap_t)
                for i, k in enumerate(act_planes):
                    nc.scalar.activation(
                        u3[:, i], v_flat,
                        mybir.ActivationFunctionType.Square,
                        bias=bias_t[:, k:k + 1], scale=1.0)
                act_set = set(act_planes)
                dve_planes = [k for k in range(D) if k not in act_set]
                for k in dve_planes:
                    nc.vector.tensor_scalar(o5[:, k], v_flat, float(k), None,
                                            op0=mybir.AluOpType.is_equal)
                for i, k in enumerate(act_planes):
                    nc.scalar.activation(
                        o5[:, k], u3[:, i],
                        mybir.ActivationFunctionType.Relu,
                        bias=1.0, scale=-1.0)

                kd = min(dve_planes[-1] + 1, min(act_planes))
                nc.sync.dma_start(o_dst[:, 0:kd], o5[:, 0:kd])
                nc.scalar.dma_start(o_dst[:, kd:D], o5[:, kd:D])
    nc.compile()
    return nc


def _run_fast(a, b, trace=False, trace_kwargs=None):
    if "fast_nc" not in _CACHE:
        _CACHE["fast_nc"] = _build_fast_nc()
    nc = _CACHE["fast_nc"]
    in_maps = []
    for cid in range(NCORES):
        sl = slice(cid * NPC, (cid + 1) * NPC)
        in_maps.append({"ab": np.ascontiguousarray(
            np.stack([a[sl], b[sl]], axis=0)).astype(np.int8)})
    res = run_bass_kernel_spmd(nc, in_maps, core_ids=list(range(NCORES)),
                               trace=trace, **(trace_kwargs or {}))
    # Gather + exact storage-format restore: device wrote k-major fp16
    # planes [10, NPC, 11]; merge to [BATCH, 11, 10] f32 (pure cast).
    out = np.empty((BATCH, W, D), dtype=np.float32)
    for cid in range(NCORES):
        sl = slice(cid * NPC, (cid + 1) * NPC)
        arr = res.results[cid]["out"]  # [10, NPC, 11] fp16
        for k in range(D):
            out[sl, :, k] = arr[k]
    return out, res


def _run_general_host(a, b, digit_table, carry_table):
    # Correctness fallback for non-structured tables (not expected from the
    # reference's setup_inputs); computed host-side.
    n = a.shape[0]
    carry = np.zeros(n, dtype=np.int64)
    out = np.empty((n, D + 1, D), dtype=digit_table.dtype)
    for p in range(D - 1, -1, -1):
        idx = carry * 100 + a[:, p].astype(np.int64) * 10 + b[:, p].astype(np.int64)
        out[:, 1 + p, :] = digit_table[idx]
        carry = np.argmax(carry_table[idx], axis=1)
    lead = np.zeros((n, D), dtype=digit_table.dtype)
    lead[np.arange(n), carry] = 1.0
    out[:, 0, :] = lead
    return out


def kernel(a, b, digit_table, carry_table):
    a = np.asarray(a, dtype=np.int32)
    b = np.asarray(b, dtype=np.int32)
    digit_table = np.asarray(digit_table, dtype=np.float32)
    carry_table = np.asarray(carry_table, dtype=np.float32)
    assert a.shape == (BATCH, D) and b.shape == (BATCH, D), (a.shape, b.shape)
    if _tables_are_structured(digit_table, carry_table):
        out, _ = _run_fast(a, b)
        return out
    return _run_general_host(a, b, digit_table, carry_table)


# revision 26
# speedup vs baseline: 1.0476x; 1.0476x over previous
"""Trainium2 Bass kernel for nn_MinimalAdderNN.

Computation (see reference): a 10-digit ripple-carry adder over base-10 digit
tensors a, b of shape [1048576, 10] (int32, digits 0..9), using two lookup
tables built deterministically by setup_inputs(). For those structured tables
the output rows are exact one-hots:
    out[n, 1+p, k] = (k == (a[n,p] + b[n,p] + carry_in) % 10)
    out[n, 0,   k] = (k == final_carry)
so the kernel computes digits/carries arithmetically on the device:
    s = a + b  ->  carry chain via one tensor_tensor_scan (pad slots reset
    state between batch elements)  ->  t = s + carry_in  ->  d = t mod 10
    ->  one-hot = is_equal(value, k) materialized as int8 {0,1} split across
    the Vector, Scalar(Act) and GpSimd engines.

The full one-hot tensor (every output element's value) is computed on
device and DMA'd out as int8; the host-side gather step only converts the
storage format (int8 0/1 -> float32 0.0/1.0, an exact cast) while
concatenating the 8 per-core shards. Writing 1 byte instead of 4 per
element cuts the dominant HBM write traffic 4x.

Sharding: pure data-parallel over batch across 8 NeuronCores (131072 rows
per core); the tables are consumed host-side only (validated against the
expected structured tables).
"""
import sys

sys.path.insert(0, "/opt/trn_rl_repo")

import numpy as np

import concourse.bacc as bacc
import concourse.mybir as mybir
import concourse.tile as tile
from concourse.bass_utils import run_bass_kernel_spmd

BATCH = 1048576
D = 10
NCORES = 8
NPC = BATCH // NCORES  # 131072 rows per core
P = 128
NQ = NPC // P          # 1024 batch elems per partition
# Variable tile sizes (batch elems per partition per tile): small head/tail
# tiles shorten pipeline fill/drain; must sum to NQ.
QS = [64, 192, 192, 192, 192, 96, 64, 32]
PW = 11                # padded slots per batch elem (10 digits + 1 pad)
W = D + 1              # output positions per batch elem (leading + 10)
OW = W * D             # 110 output bytes per batch elem

# Engine split of the one-hot work over the 11 output positions w
# (w=0 leading carry, w=1..10 digits MSD..LSD). Each engine materializes
# o[:, :, w_range, k_range] = is_equal(val[w], k).
# One-hot lanes k are computed as whole planes: plane k holds
# [val[s,w] == k] for all (s,w). Output HBM layout is k-major
# [10, NPC, 11] fp16; the host merges planes into [n, w, k] f32
# (exact cast). Plane list split between DVE (4x tensor_scalar
# is_equal) and ScalarE (2-pass square/relu, fully contiguous).
ACT_PLANES = (7, 8, 9)     # k-planes computed by ScalarE
ALT_PLANE = 6              # ... plus this one on odd tiles

f32 = mybir.dt.float32
f16 = mybir.dt.float16
i8 = mybir.dt.int8

_CACHE = {}


def _expected_tables():
    next_carry = np.zeros((200, 2), dtype=np.float32)
    digit = np.zeros((200, 10), dtype=np.float32)
    for carry in (0, 1):
        for a_ in range(10):
            for b_ in range(10):
                idx = carry * 100 + a_ * 10 + b_
                total = a_ + b_ + carry
                next_carry[idx, total // 10] = 1.0
                digit[idx, total % 10] = 1.0
    return digit, next_carry


def _tables_are_structured(digit_table, carry_table):
    digit_exp, carry_exp = _expected_tables()
    if digit_table.shape != (200, 10) or carry_table.shape != (200, 2):
        return False
    if not np.array_equal(digit_table, digit_exp):
        return False
    # The reference only consumes argmax(carry_table[idx]); the fast path is
    # valid iff that argmax equals the arithmetic carry bit for every index.
    bits = np.argmax(carry_table, axis=1)
    return np.array_equal(bits, np.argmax(carry_exp, axis=1))


def _build_fast_nc():
    assert sum(QS) == NQ
    qmax = max(QS)
    nc = bacc.Bacc()
    ab_d = nc.dram_tensor("ab", [2, NPC, D], i8, kind="ExternalInput").ap()
    o_d = nc.dram_tensor("out", [D, NPC, W], f16, kind="ExternalOutput").ap()

    with tile.TileContext(nc) as tc:
        with tc.tile_pool(name="const", bufs=1) as cp, \
             tc.tile_pool(name="io", bufs=2) as iop, \
             tc.tile_pool(name="wk", bufs=2) as wp, \
             tc.tile_pool(name="ot", bufs=3) as op_:
            tens = cp.tile([P, qmax * PW], f16, tag="tens")
            nc.vector.memset(tens[:], 10.0)
            bias_t = cp.tile([P, D], f32, tag="bias")
            for k in range(D):
                nc.vector.memset(bias_t[:, k:k + 1], -float(k))

            r0 = 0
            for t_i, Q in enumerate(QS):
                ab_src = ab_d[:, r0:r0 + P * Q, :] \
                    .rearrange("z (p q) d -> p z (q d)", q=Q)
                o_dst = o_d[:, r0:r0 + P * Q, :] \
                    .rearrange("k (p q) w -> p k (q w)", q=Q)
                r0 += P * Q

                # ---- load (SWDGE cast int8 -> fp16 on the gpsimd ring)
                abt = iop.tile([P, 2 * qmax * D], f16, tag="ab")
                nc.gpsimd.dma_start(
                    abt[:, :2 * Q * D].rearrange("p (z f) -> p z f", z=2),
                    ab_src)
                ab4 = abt[:, :2 * Q * D].rearrange("p (z q d) -> p z q d",
                                                   z=2, d=D)
                a3 = ab4[:, 0]
                b3 = ab4[:, 1]

                # ---- s[q, j] = a[q, 9-j] + b[q, 9-j]  (LSD first), pads 0
                s_pad = wp.tile([P, qmax * PW], f16, tag="s")
                sp3 = s_pad[:, :Q * PW].rearrange("p (q e) -> p q e", e=PW)
                nc.gpsimd.memset(sp3[:, :, D:PW], 0.0)
                nc.gpsimd.tensor_tensor(sp3[:, :, 0:D], a3[:, :, ::-1],
                                        b3[:, :, ::-1], op=mybir.AluOpType.add)

                # ---- carry chain: scan written shifted by +1 so that
                # c3[q, j] = carry INTO digit j; c3[q, 10] = final carry.
                c = wp.tile([P, qmax * PW + 2], f16, tag="c")
                nc.gpsimd.memset(c[:, 0:1], 0.0)
                nc.vector.tensor_tensor_scan(
                    c[:, 1:1 + Q * PW], s_pad[:, :Q * PW],
                    tens[:, :Q * PW], 0.0,
                    op0=mybir.AluOpType.add, op1=mybir.AluOpType.is_ge)
                c3 = c[:, 0:Q * PW].rearrange("p (q e) -> p q e", e=PW)

                # ---- carry correction, flat+forward on DVE:
                # w[q,j] = carry_in(j) - 10*carry_out(j)  (j=0..9)
                # w[q,10] = final carry (c3[q,10] - 10*next-sample pad 0)
                cw = wp.tile([P, qmax * PW + 2], f16, tag="w")
                nc.vector.scalar_tensor_tensor(
                    cw[:, :Q * PW], c[:, 1:1 + Q * PW], -10.0,
                    c[:, 0:Q * PW],
                    op0=mybir.AluOpType.mult, op1=mybir.AluOpType.add)
                cw3 = cw[:, :Q * PW].rearrange("p (q e) -> p q e", e=PW)

                # ---- v[q, w] = value at output position w, CONTIGUOUS:
                #      w=0 -> final carry; w>=1 -> digit j=10-w:
                #      s[q,j] + carry_in(j) - 10*carry_out(j)
                tt = wp.tile([P, qmax * W], f16, tag="t")
                t3 = tt[:, :Q * W].rearrange("p (q e) -> p q e", e=W)
                nc.gpsimd.tensor_tensor(t3, sp3[:, :, 0:W][:, :, ::-1],
                                        cw3[:, :, 0:W][:, :, ::-1],
                                        op=mybir.AluOpType.add)

                v_flat = tt[:, :Q * W]

                # ---- one-hot planes
                ot = op_.tile([P, qmax * W * D], f16, tag="o")
                o5 = ot[:, :Q * W * D].rearrange("p (k f) -> p k f", k=D)

                act_planes = list(ACT_PLANES)
                nap_t = len(act_planes)
                u = wp.tile([P, qmax * W * len(ACT_PLANES)], f16,
                            tag="u")
                u3 = u[:, :Q * W * nap_t].rearrange("p (k f) -> p k f",
                                                    k=nap_t)
                for i, k in enumerate(act_planes):
                    nc.scalar.activation(
                        u3[:, i], v_flat,
                        mybir.ActivationFunctionType.Square,
                        bias=bias_t[:, k:k + 1], scale=1.0)
                act_set = set(act_planes)
                dve_planes = [k for k in range(D) if k not in act_set]
                for k in dve_planes:
                    nc.vector.tensor_scalar(o5[:, k], v_flat, float(k), None,
                                            op0=mybir.AluOpType.is_equal)
                for i, k in enumerate(act_planes):
                    nc.scalar.activation(
                        o5[:, k], u3[:, i],
                        mybir.ActivationFunctionType.Relu,
                        bias=1.0, scale=-1.0)

                kd = min(dve_planes[-1] + 1, min(act_planes))
                nc.sync.dma_start(o_dst[:, 0:kd], o5[:, 0:kd])
                nc.scalar.dma_start(o_dst[:, kd:D], o5[:, kd:D])
    nc.compile()
    return nc


def _run_fast(a, b, trace=False, trace_kwargs=None):
    if "fast_nc" not in _CACHE:
        _CACHE["fast_nc"] = _build_fast_nc()
    nc = _CACHE["fast_nc"]
    in_maps = []
    for cid in range(NCORES):
        sl = slice(cid * NPC, (cid + 1) * NPC)
        in_maps.append({"ab": np.ascontiguousarray(
            np.stack([a[sl], b[sl]], axis=0)).astype(np.int8)})
    res = run_bass_kernel_spmd(nc, in_maps, core_ids=list(range(NCORES)),
                               trace=trace, **(trace_kwargs or {}))
    # Gather + exact storage-format restore: device wrote k-major fp16
    # planes [10, NPC, 11]; merge to [BATCH, 11, 10] f32 (pure cast).
    out = np.empty((BATCH, W, D), dtype=np.float32)
    for cid in range(NCORES):
        sl = slice(cid * NPC, (cid + 1) * NPC)
        arr = res.results[cid]["out"]  # [10, NPC, 11] fp16
        for k in range(D):
            out[sl, :, k] = arr[k]
    return out, res


def _run_general_host(a, b, digit_table, carry_table):
    # Correctness fallback for non-structured tables (not expected from the
    # reference's setup_inputs); computed host-side.
    n = a.shape[0]
    carry = np.zeros(n, dtype=np.int64)
    out = np.empty((n, D + 1, D), dtype=digit_table.dtype)
    for p in range(D - 1, -1, -1):
        idx = carry * 100 + a[:, p].astype(np.int64) * 10 + b[:, p].astype(np.int64)
        out[:, 1 + p, :] = digit_table[idx]
        carry = np.argmax(carry_table[idx], axis=1)
    lead = np.zeros((n, D), dtype=digit_table.dtype)
    lead[np.arange(n), carry] = 1.0
    out[:, 0, :] = lead
    return out


def kernel(a, b, digit_table, carry_table):
    a = np.asarray(a, dtype=np.int32)
    b = np.asarray(b, dtype=np.int32)
    digit_table = np.asarray(digit_table, dtype=np.float32)
    carry_table = np.asarray(carry_table, dtype=np.float32)
    assert a.shape == (BATCH, D) and b.shape == (BATCH, D), (a.shape, b.shape)
    if _tables_are_structured(digit_table, carry_table):
        out, _ = _run_fast(a, b)
        return out
    return _run_general_host(a, b, digit_table, carry_table)


# revision 28
# speedup vs baseline: 1.0817x; 1.0325x over previous
"""Trainium2 Bass kernel for nn_MinimalAdderNN.

Computation (see reference): a 10-digit ripple-carry adder over base-10 digit
tensors a, b of shape [1048576, 10] (int32, digits 0..9), using two lookup
tables built deterministically by setup_inputs(). For those structured tables
the output rows are exact one-hots:
    out[n, 1+p, k] = (k == (a[n,p] + b[n,p] + carry_in) % 10)
    out[n, 0,   k] = (k == final_carry)
so the kernel computes digits/carries arithmetically on the device:
    s = a + b  ->  carry chain via one tensor_tensor_scan (pad slots reset
    state between batch elements)  ->  t = s + carry_in  ->  d = t mod 10
    ->  one-hot = is_equal(value, k) materialized as int8 {0,1} split across
    the Vector, Scalar(Act) and GpSimd engines.

The full one-hot tensor (every output element's value) is computed on
device and DMA'd out as int8; the host-side gather step only converts the
storage format (int8 0/1 -> float32 0.0/1.0, an exact cast) while
concatenating the 8 per-core shards. Writing 1 byte instead of 4 per
element cuts the dominant HBM write traffic 4x.

Sharding: pure data-parallel over batch across 8 NeuronCores (131072 rows
per core); the tables are consumed host-side only (validated against the
expected structured tables).
"""
import sys

sys.path.insert(0, "/opt/trn_rl_repo")

import numpy as np

import concourse.bacc as bacc
import concourse.mybir as mybir
import concourse.tile as tile
from concourse.bass_utils import run_bass_kernel_spmd

BATCH = 1048576
D = 10
NCORES = 8
NPC = BATCH // NCORES  # 131072 rows per core
P = 128
NQ = NPC // P          # 1024 batch elems per partition
# Variable tile sizes (batch elems per partition per tile): small head/tail
# tiles shorten pipeline fill/drain; must sum to NQ.
QS = [32, 64, 160, 160, 160, 160, 128, 96, 64]
PW = 11                # padded slots per batch elem (10 digits + 1 pad)
W = D + 1              # output positions per batch elem (leading + 10)
OW = W * D             # 110 output bytes per batch elem

# Engine split of the one-hot work over the 11 output positions w
# (w=0 leading carry, w=1..10 digits MSD..LSD). Each engine materializes
# o[:, :, w_range, k_range] = is_equal(val[w], k).
# One-hot lanes k are computed as whole planes: plane k holds
# [val[s,w] == k] for all (s,w). Output HBM layout is k-major
# [10, NPC, 11] fp16; the host merges planes into [n, w, k] f32
# (exact cast). Plane list split between DVE (4x tensor_scalar
# is_equal) and ScalarE (2-pass square/relu, fully contiguous).
ACT_PLANES = (7, 8, 9)     # k-planes computed by ScalarE
ALT_PLANE = 6              # ... plus this one on odd tiles

f32 = mybir.dt.float32
f16 = mybir.dt.float16
i8 = mybir.dt.int8

_CACHE = {}


def _expected_tables():
    next_carry = np.zeros((200, 2), dtype=np.float32)
    digit = np.zeros((200, 10), dtype=np.float32)
    for carry in (0, 1):
        for a_ in range(10):
            for b_ in range(10):
                idx = carry * 100 + a_ * 10 + b_
                total = a_ + b_ + carry
                next_carry[idx, total // 10] = 1.0
                digit[idx, total % 10] = 1.0
    return digit, next_carry


def _tables_are_structured(digit_table, carry_table):
    digit_exp, carry_exp = _expected_tables()
    if digit_table.shape != (200, 10) or carry_table.shape != (200, 2):
        return False
    if not np.array_equal(digit_table, digit_exp):
        return False
    # The reference only consumes argmax(carry_table[idx]); the fast path is
    # valid iff that argmax equals the arithmetic carry bit for every index.
    bits = np.argmax(carry_table, axis=1)
    return np.array_equal(bits, np.argmax(carry_exp, axis=1))


def _build_fast_nc():
    assert sum(QS) == NQ
    qmax = max(QS)
    nc = bacc.Bacc()
    ab_d = nc.dram_tensor("ab", [2, NPC, D], i8, kind="ExternalInput").ap()
    o_d = nc.dram_tensor("out", [D, NPC, W], f16, kind="ExternalOutput").ap()

    with tile.TileContext(nc) as tc:
        with tc.tile_pool(name="const", bufs=1) as cp, \
             tc.tile_pool(name="io", bufs=2) as iop, \
             tc.tile_pool(name="wk", bufs=2) as wp, \
             tc.tile_pool(name="ot", bufs=3) as op_:
            tens = cp.tile([P, qmax * PW], f16, tag="tens")
            nc.vector.memset(tens[:], 10.0)
            bias_t = cp.tile([P, D], f32, tag="bias")
            for k in range(D):
                nc.vector.memset(bias_t[:, k:k + 1], -float(k))

            r0 = 0
            for t_i, Q in enumerate(QS):
                ab_src = ab_d[:, r0:r0 + P * Q, :] \
                    .rearrange("z (p q) d -> p z (q d)", q=Q)
                o_dst = o_d[:, r0:r0 + P * Q, :] \
                    .rearrange("k (p q) w -> p k (q w)", q=Q)
                r0 += P * Q

                # ---- load (SWDGE cast int8 -> fp16 on the gpsimd ring)
                abt = iop.tile([P, 2 * qmax * D], f16, tag="ab")
                nc.gpsimd.dma_start(
                    abt[:, :2 * Q * D].rearrange("p (z f) -> p z f", z=2),
                    ab_src)
                ab4 = abt[:, :2 * Q * D].rearrange("p (z q d) -> p z q d",
                                                   z=2, d=D)
                a3 = ab4[:, 0]
                b3 = ab4[:, 1]

                # ---- s[q, j] = a[q, 9-j] + b[q, 9-j]  (LSD first), pads 0
                s_pad = wp.tile([P, qmax * PW], f16, tag="s")
                sp3 = s_pad[:, :Q * PW].rearrange("p (q e) -> p q e", e=PW)
                nc.gpsimd.memset(sp3[:, :, D:PW], 0.0)
                nc.gpsimd.tensor_tensor(sp3[:, :, 0:D], a3[:, :, ::-1],
                                        b3[:, :, ::-1], op=mybir.AluOpType.add)

                # ---- carry chain: scan written shifted by +1 so that
                # c3[q, j] = carry INTO digit j; c3[q, 10] = final carry.
                c = wp.tile([P, qmax * PW + 2], f16, tag="c")
                nc.gpsimd.memset(c[:, 0:1], 0.0)
                nc.vector.tensor_tensor_scan(
                    c[:, 1:1 + Q * PW], s_pad[:, :Q * PW],
                    tens[:, :Q * PW], 0.0,
                    op0=mybir.AluOpType.add, op1=mybir.AluOpType.is_ge)
                c3 = c[:, 0:Q * PW].rearrange("p (q e) -> p q e", e=PW)

                # ---- carry correction, flat+forward on DVE:
                # w[q,j] = carry_in(j) - 10*carry_out(j)  (j=0..9)
                # w[q,10] = final carry (c3[q,10] - 10*next-sample pad 0)
                cw = wp.tile([P, qmax * PW + 2], f16, tag="w")
                nc.vector.scalar_tensor_tensor(
                    cw[:, :Q * PW], c[:, 1:1 + Q * PW], -10.0,
                    c[:, 0:Q * PW],
                    op0=mybir.AluOpType.mult, op1=mybir.AluOpType.add)
                cw3 = cw[:, :Q * PW].rearrange("p (q e) -> p q e", e=PW)

                # ---- v[q, w] = value at output position w, CONTIGUOUS:
                #      w=0 -> final carry; w>=1 -> digit j=10-w:
                #      s[q,j] + carry_in(j) - 10*carry_out(j)
                tt = wp.tile([P, qmax * W], f16, tag="t")
                t3 = tt[:, :Q * W].rearrange("p (q e) -> p q e", e=W)
                nc.gpsimd.tensor_tensor(t3, sp3[:, :, 0:W][:, :, ::-1],
                                        cw3[:, :, 0:W][:, :, ::-1],
                                        op=mybir.AluOpType.add)

                v_flat = tt[:, :Q * W]

                # ---- one-hot planes
                ot = op_.tile([P, qmax * W * D], f16, tag="o")
                o5 = ot[:, :Q * W * D].rearrange("p (k f) -> p k f", k=D)

                act_planes = list(ACT_PLANES)
                if 2 <= t_i <= 5:
                    act_planes.append(ALT_PLANE)
                nap_t = len(act_planes)
                u = wp.tile([P, qmax * W * (len(ACT_PLANES) + 1)], f16,
                            tag="u")
                u3 = u[:, :Q * W * nap_t].rearrange("p (k f) -> p k f",
                                                    k=nap_t)
                for i, k in enumerate(act_planes):
                    nc.scalar.activation(
                        u3[:, i], v_flat,
                        mybir.ActivationFunctionType.Square,
                        bias=bias_t[:, k:k + 1], scale=1.0)
                act_set = set(act_planes)
                dve_planes = [k for k in range(D) if k not in act_set]
                for k in dve_planes:
                    nc.vector.tensor_scalar(o5[:, k], v_flat, float(k), None,
                                            op0=mybir.AluOpType.is_equal)
                for i, k in enumerate(act_planes):
                    nc.scalar.activation(
                        o5[:, k], u3[:, i],
                        mybir.ActivationFunctionType.Relu,
                        bias=1.0, scale=-1.0)

                kd = min(dve_planes[-1] + 1, min(act_planes))
                nc.sync.dma_start(o_dst[:, 0:kd], o5[:, 0:kd])
                nc.scalar.dma_start(o_dst[:, kd:D], o5[:, kd:D])
    nc.compile()
    return nc


def _run_fast(a, b, trace=False, trace_kwargs=None):
    if "fast_nc" not in _CACHE:
        _CACHE["fast_nc"] = _build_fast_nc()
    nc = _CACHE["fast_nc"]
    in_maps = []
    for cid in range(NCORES):
        sl = slice(cid * NPC, (cid + 1) * NPC)
        in_maps.append({"ab": np.ascontiguousarray(
            np.stack([a[sl], b[sl]], axis=0)).astype(np.int8)})
    res = run_bass_kernel_spmd(nc, in_maps, core_ids=list(range(NCORES)),
                               trace=trace, **(trace_kwargs or {}))
    # Gather + exact storage-format restore: device wrote k-major fp16
    # planes [10, NPC, 11]; merge to [BATCH, 11, 10] f32 (pure cast).
    out = np.empty((BATCH, W, D), dtype=np.float32)
    for cid in range(NCORES):
        sl = slice(cid * NPC, (cid + 1) * NPC)
        arr = res.results[cid]["out"]  # [10, NPC, 11] fp16
        for k in range(D):
            out[sl, :, k] = arr[k]
    return out, res


def _run_general_host(a, b, digit_table, carry_table):
    # Correctness fallback for non-structured tables (not expected from the
    # reference's setup_inputs); computed host-side.
    n = a.shape[0]
    carry = np.zeros(n, dtype=np.int64)
    out = np.empty((n, D + 1, D), dtype=digit_table.dtype)
    for p in range(D - 1, -1, -1):
        idx = carry * 100 + a[:, p].astype(np.int64) * 10 + b[:, p].astype(np.int64)
        out[:, 1 + p, :] = digit_table[idx]
        carry = np.argmax(carry_table[idx], axis=1)
    lead = np.zeros((n, D), dtype=digit_table.dtype)
    lead[np.arange(n), carry] = 1.0
    out[:, 0, :] = lead
    return out


def kernel(a, b, digit_table, carry_table):
    a = np.asarray(a, dtype=np.int32)
    b = np.asarray(b, dtype=np.int32)
    digit_table = np.asarray(digit_table, dtype=np.float32)
    carry_table = np.asarray(carry_table, dtype=np.float32)
    assert a.shape == (BATCH, D) and b.shape == (BATCH, D), (a.shape, b.shape)
    if _tables_are_structured(digit_table, carry_table):
        out, _ = _run_fast(a, b)
        return out
    return _run_general_host(a, b, digit_table, carry_table)
